# revision 18
# baseline (speedup 1.0000x reference)
"""AdaConv2d on 8 TRN2 NeuronCores.

Per-sample adaptive 3x3 conv (stride 1, pad 1): each sample b uses
kernel_base * kernel_mask[demog_label[b]].

Strategy: data-parallel over batch (8 samples/core). Host gathers the
per-sample mask (by label) and pre-pads x; device computes the per-sample
masked kernel (9 tensor_scalar_mul) and the conv as 9 shifted bf16 matmuls
(K=IC=128, M=128 oc-block, N=448 pixels) accumulating in PSUM.
"""

import numpy as np
from ml_dtypes import bfloat16

NCORES = 8
BS = 8            # samples per core
IC, OC, KS = 128, 256, 3
H = W = 56
HP = WP = 58      # padded
NPIX = H * W      # 3136
RROWS = 8         # output rows per matmul tile
RT = H // RROWS   # 7 row-tiles
NTAP = KS * KS    # 9
NFREE = RROWS * W # 448

_cached_nc = None


def _build():
    import concourse.mybir as mybir
    import concourse.bacc as bacc
    import concourse.tile as tile

    nc = bacc.Bacc("TRN2", target_bir_lowering=False, debug=False)
    bf = mybir.dt.bfloat16
    f32 = mybir.dt.float32

    x_ext = nc.declare_dram_parameter("x", [BS, IC, HP, WP], bf, isOutput=False)
    kb_ext = nc.declare_dram_parameter("kb", [IC, NTAP, OC], bf, isOutput=False)
    mk_ext = nc.declare_dram_parameter("mk", [IC, BS * NTAP], f32, isOutput=False)
    out_ext = nc.declare_dram_parameter("out", [BS, 2, 128, NPIX], f32, isOutput=True)

    with tile.TileContext(nc) as tc:
        with (
            tc.tile_pool(name="const", bufs=1) as cpool,
            tc.tile_pool(name="xin", bufs=3) as xpool,
            tc.tile_pool(name="wgt", bufs=2) as wpool,
            tc.tile_pool(name="ostage", bufs=3) as opool,
            tc.tile_pool(name="psum", bufs=7, space="PSUM") as pspool,
        ):
            # PE warmup: ~4us of dummy matmuls with no input deps so the HAM
            # clock-gate reaches 8/8 before the first real matmul.
            wub = cpool.tile([IC, 448], bf)
            nc.vector.memset(wub[:], 0.0)
            wps = pspool.tile([128, 448], f32, name="wups", tag="wups", bufs=1)
            for _ in range(12):
                nc.tensor.matmul(wps[:], wub[:, :128], wub[:], start=True, stop=True)

            # x[0] first: its transfer gates the first real matmul stream.
            # Split so the first row-tiles' rows arrive before the full image.
            xp0 = xpool.tile([IC, HP, WP], bf, name="xp0", tag="xp")
            nc.sync.dma_start(xp0[:, :18, :], x_ext[0, :, :18, :])
            nc.sync.dma_start(xp0[:, 18:, :], x_ext[0, :, 18:, :])
            mk = cpool.tile([IC, BS * NTAP], f32)
            nc.sync.dma_start(mk[:], mk_ext[:])
            kb = cpool.tile([IC, NTAP, OC], bf)
            for c0, c1 in ((0, 1), (1, 5), (5, 9)):
                nc.sync.dma_start(kb[:, c0:c1, :], kb_ext[:, c0:c1, :])

            for s in range(BS):
                if s == 0:
                    xp = xp0
                else:
                    xp = xpool.tile([IC, HP, WP], bf, name=f"xp{s}", tag="xp")
                    nc.sync.dma_start(xp[:], x_ext[s])

                w = wpool.tile([IC, NTAP, OC], bf, name=f"w{s}", tag="w")
                for t in range(NTAP):
                    # sample 0 gates the stream start: split its prep across
                    # two engines so taps are produced ~2x faster than the
                    # matmul stream consumes them.
                    eng = nc.gpsimd if (s == 0 and t % 2 == 1) else nc.vector
                    eng.tensor_scalar_mul(
                        w[:, t, :], kb[:, t, :],
                        mk[:, s * NTAP + t : s * NTAP + t + 1],
                    )

                for ocb in range(2):
                    ost = opool.tile([128, NPIX], f32, name=f"ost{s}_{ocb}", tag="ost")
                    for rt in range(RT):
                        ps = pspool.tile([128, NFREE], f32, name=f"ps{s}_{ocb}_{rt}", tag="ps")
                        for t in range(NTAP):
                            kh, kw = divmod(t, KS)
                            rhs = xp[:, rt * RROWS + kh : rt * RROWS + kh + RROWS, kw : kw + W]
                            nc.tensor.matmul(
                                ps[:],
                                w[:, t, ocb * 128 : (ocb + 1) * 128],
                                rhs,
                                start=(t == 0),
                                stop=(t == NTAP - 1),
                            )
                        nc.any.tensor_copy(ost[:, rt * NFREE : (rt + 1) * NFREE], ps[:])
                        # chunked output DMA: each row-tile ships as soon as its
                        # drain lands, so the kernel tail is one small DMA.
                        nc.sync.dma_start(
                            out_ext[s, ocb, :, rt * NFREE : (rt + 1) * NFREE],
                            ost[:, rt * NFREE : (rt + 1) * NFREE],
                        )
    nc.compile()
    return nc


def run(inputs, trace=False, **kw):
    from concourse.bass_utils import run_bass_kernel_spmd

    global _cached_nc
    if _cached_nc is None:
        _cached_nc = _build()
    nc = _cached_nc

    x = np.asarray(inputs["x"])
    demog_label = np.asarray(inputs["demog_label"])
    kernel_base = np.asarray(inputs["kernel_base"])
    kernel_mask = np.asarray(inputs["kernel_mask"])
    B = x.shape[0]

    xpad = np.zeros((B, IC, HP, WP), dtype=bfloat16)
    xpad[:, :, 1 : H + 1, 1 : W + 1] = x.astype(bfloat16)

    # [OC, IC, 3, 3] -> [IC, 9, OC]
    kb_t = np.ascontiguousarray(
        kernel_base.transpose(1, 2, 3, 0).reshape(IC, NTAP, OC)
    ).astype(bfloat16)
    # gather per-sample mask by label: [B, IC, 9]
    mask_t = kernel_mask[demog_label].reshape(B, IC, NTAP).astype(np.float32)

    in_maps = []
    for c in range(NCORES):
        sl = slice(c * BS, (c + 1) * BS)
        mk_c = np.ascontiguousarray(
            mask_t[sl].transpose(1, 0, 2).reshape(IC, BS * NTAP)
        )
        in_maps.append({
            "x": np.ascontiguousarray(xpad[sl]),
            "kb": kb_t,
            "mk": mk_c,
        })

    res = None
    last_exc = None
    for _attempt in range(3):
        try:
            res = run_bass_kernel_spmd(nc, in_maps, core_ids=list(range(NCORES)),
                                       trace=trace, **kw)
            break
        except Exception as e:  # transient NRT/device faults: retry
            last_exc = e
            import time
            time.sleep(5)
    if res is None:
        raise last_exc
    outs = [r["out"].reshape(BS, OC, H, W) for r in res.results]
    full = np.concatenate(outs, axis=0).astype(np.float32)
    return full, res


def kernel(**inputs):
    out, _ = run(inputs, trace=False)
    return out
